# revision 1
# baseline (speedup 1.0000x reference)
"""Trainium2 Bass kernel v2 for 3rd-order HONU (nn_HONU_80865644249720).

out[b] = sum_{i<=j<=k} w_{ijk} xb_i xb_j xb_k,  xb = [1, x] (65 feats)

Squaring trick with the x^2 correction FOLDED into the pair weights:
  ss_p = (x_j + x_k)^2  for every pair j<=k;  ss_(m,m) = 4 x_m^2, so the
  -0.5(x_j^2+x_k^2)W[p] correction rows land on diagonal-pair weights.
  The 65 bias pairs (0,k) carry only linear/const monomials and are
  evaluated on the host, as are 96 residual off-diag pairs; the 64
  diagonal pairs are replicated into both halves so each half is
  EXACTLY 1024 pair columns = 8 tiles of 128 (and sel K drops to 64).

Per core (4-way batch x 2-way pairs, BC=512):
  sel_t : ps[128p,512b] = es_t^T @ xbt           (PE, bf16, 8x)
  sq_t  : ss = ps^2 -> bf16 SBUF                 (ACT/DVE/Pool split)
  wf_t,c: ztT_c[128b,65] += ss[:,c128]^T @ wh_t  (PE, bf16, 32x small)
  dump  : ztT [128, 4*65] fp32 -> SBUF -> DRAM; host does
          out[b] = sum_i ztT[b,i] * xb[b,i] and the half-sum.
"""

import os

import numpy as np

IN_FEATURES = 64
NF = IN_FEATURES + 1  # 65 features incl. bias
BATCH = 2048
N_CORES = 8
NBS = 4  # batch shards
NPS = 2  # pair shards
BC = BATCH // NBS  # 512 batch rows per core
PH = 8  # pair tiles per half
PCOLS = PH * 128  # 1024 pair columns per half (fully used)
NCH = 4  # batch chunks for the flipped W matmuls
CHB = BC // NCH  # 128
# square engine per tile: a=ACT, v=DVE, p=Pool
SQ_ENG = "PaPavava"

_CACHE = {}
LAST_RESULTS = None


def _build_bass():
    import concourse.bacc as bacc
    import concourse.mybir as mybir
    import concourse.tile as tile

    f32 = mybir.dt.float32
    bf16 = mybir.dt.float16
    Square = mybir.ActivationFunctionType.Square
    Copy = mybir.ActivationFunctionType.Copy

    nc = bacc.Bacc(
        target_bir_lowering=False,
        debug=False,
        enable_asserts=False,
        num_devices=N_CORES,
    )

    # blob1 = [xbt | es tiles 0-1], blob2 = [es tiles 2-8], blob3 = wh tiles
    a_d = nc.dram_tensor("a", [64, BC + 2 * 128], bf16, kind="ExternalInput").ap()
    b_d = nc.dram_tensor("b", [64, 6 * 128], bf16, kind="ExternalInput").ap()
    c_d = nc.dram_tensor("c", [128, PH * NF], bf16, kind="ExternalInput").ap()
    out_d = nc.dram_tensor("out", [128, NCH * NF], bf16, kind="ExternalOutput").ap()

    from contextlib import ExitStack

    with tile.TileContext(nc) as tc, ExitStack() as ctx:
        consts = ctx.enter_context(tc.tile_pool(name="consts", bufs=1))
        ss_pool = ctx.enter_context(tc.tile_pool(name="ss", bufs=9))
        psc_pool = ctx.enter_context(tc.tile_pool(name="psc", bufs=5))
        ps_pool = ctx.enter_context(tc.tile_pool(name="ps", bufs=7, space="PSUM"))
        zt_pool = ctx.enter_context(tc.tile_pool(name="zt", bufs=1, space="PSUM"))

        a = consts.tile([64, BC + 2 * 128], bf16, tag="a")
        b = consts.tile([64, 6 * 128], bf16, tag="b")
        c = consts.tile([128, PH * NF], bf16, tag="c")
        nc.sync.dma_start(a[:], a_d)
        nc.sync.dma_start(b[:], b_d)
        nc.sync.dma_start(c[:], c_d)

        xbt = a[:, 0:BC]
        ztT = zt_pool.tile([128, NCH * NF], f32)

        def es_tile(t):
            if t < 2:
                return a[:, BC + t * 128 : BC + (t + 1) * 128]
            return b[:, (t - 2) * 128 : (t - 1) * 128]

        ps_tiles = [None] * PH
        ss_tiles = [None] * PH
        deferred_act = []

        def emit_sel(t):
            ps = ps_pool.tile([128, BC], f32, tag="ps")
            nc.tensor.matmul(ps[:], es_tile(t), xbt)
            ps_tiles[t] = ps
            ss = ss_pool.tile([128, BC], bf16, tag="ss")
            eng = SQ_ENG[t]
            if eng == "a":
                nc.scalar.activation(ss[:], ps[:], Square)
            else:
                psc = psc_pool.tile([128, BC], bf16, tag="psc")
                if eng == "c":
                    nc.scalar.activation(psc[:], ps[:], Copy)
                else:
                    nc.vector.tensor_copy(psc[:], ps[:])
                if eng == "h":
                    hb = BC // 2
                    nc.vector.tensor_mul(ss[:, 0:hb], psc[:, 0:hb], psc[:, 0:hb])
                    deferred_act.append((ss, psc, hb))
                elif eng in ("v", "c"):
                    nc.vector.tensor_mul(ss[:], psc[:], psc[:])
                else:
                    nc.gpsimd.tensor_mul(ss[:], psc[:], psc[:])
            ss_tiles[t] = ss

        def emit_wf(t):
            ss = ss_tiles[t]
            for ch in range(NCH):
                nc.tensor.matmul(
                    ztT[:, ch * NF : (ch + 1) * NF],
                    ss[:, ch * CHB : (ch + 1) * CHB],
                    c[:, t * NF : (t + 1) * NF],
                    start=(t == WF_ORDER[0] and ch == 0),
                    stop=(t == WF_ORDER[-1]),
                    skip_group_check=True,
                )

        # PE order: 6 sels up front, then wf groups ordered by expected
        # square readiness. DVE (longest egress pole) takes tile 0 so it
        # starts at sel0; ACT squares t1,t3,t5,t7,t8; Pool t0,t2 via copies.
        WF_ORDER = [1, 3, 0, 5, 7, 4, 2, 6]
        for t in range(6):
            emit_sel(t)
        for i, t in enumerate(WF_ORDER):
            if i < 2:
                emit_sel(i + 6)
            if i == 2:
                for ss_d, psc_d, hb in deferred_act:
                    nc.scalar.activation(ss_d[:, hb:], psc_d[:, hb:], Square)
            emit_wf(t)

        # dump ztT: PSUM -> SBUF (bf16) -> DRAM
        outsb = consts.tile([128, NCH * NF], bf16, tag="outsb")
        nc.vector.tensor_copy(outsb[:], ztT[:])
        nc.sync.dma_start(out_d, outsb[:])

    nc.compile()
    return nc


def _pair_maps():
    jp = np.concatenate([np.full(NF - j, j, np.int64) for j in range(NF)])
    kp = np.concatenate([np.arange(j, NF, dtype=np.int64) for j in range(NF)])
    return jp, kp


def _host_prep(x, weights, comb_idx):
    """Build per-core bf16 input blobs (numpy only)."""
    import ml_dtypes

    bf16 = np.float16
    jp, kp = _pair_maps()
    npair = len(jp)  # 2145

    ci = np.asarray(comb_idx, np.int64)
    c0, c1, c2 = ci[:, 0], ci[:, 1], ci[:, 2]
    pcol = c1 * NF - (c1 * (c1 - 1)) // 2 + (c2 - c1)
    w2 = np.zeros((npair, NF), np.float32)  # [pair, i]
    w2[pcol, c0] = np.asarray(weights, np.float32)

    xb = np.concatenate(
        [np.ones((BATCH, 1), np.float32), np.asarray(x, np.float32)], axis=1
    )

    # host-side terms: bias pairs (0,k) carry only linear/const monomials;
    # plus 48 residual off-diag pairs per half so each half is exactly 1024.
    bias_pairs = np.where(jp == 0)[0]
    host_out = (xb @ w2[bias_pairs, 0]).astype(np.float64)
    dev = np.where(jp >= 1)[0]
    diag_idx = dev[jp[dev] == kp[dev]]  # 64 diag pairs, feature order
    off_idx = dev[jp[dev] != kp[dev]]  # 2016
    offA, offB = off_idx[:1008], off_idx[1008:]
    rem = np.concatenate([offA[-48:], offB[-48:]])
    wps = xb @ w2[rem].T
    host_out += np.sum(
        wps * xb[:, jp[rem]] * xb[:, kp[rem]], axis=1, dtype=np.float64
    )
    off_h = [offA[:-48], offB[:-48]]

    es_h, wh_h = [], []
    for h in range(NPS):
        offs = off_h[h]
        pidx = np.concatenate([diag_idx, offs])
        pcount = len(pidx)  # 1024 exactly
        es = np.zeros((64, PCOLS), np.float32)  # feature f -> row f-1
        np.add.at(es, (jp[pidx] - 1, np.arange(pcount)), 1.0)
        np.add.at(es, (kp[pidx] - 1, np.arange(pcount)), 1.0)
        wh = np.zeros((PCOLS, NF), np.float32)
        wh[64:pcount] = 0.5 * w2[offs]
        if h == 0:
            wh[:64] = 0.25 * w2[diag_idx]  # ss_diag = 4 x^2
        np.add.at(wh, jp[offs] - 1, -0.125 * w2[offs])
        np.add.at(wh, kp[offs] - 1, -0.125 * w2[offs])
        es_h.append(es.astype(bf16))
        # wh tiles: [128, PH*NF], tile t = wh[t*128:(t+1)*128, :]
        wh_h.append(
            np.ascontiguousarray(
                wh.reshape(PH, 128, NF).transpose(1, 0, 2).reshape(128, PH * NF)
            ).astype(bf16)
        )

    xbt_q = []
    for q in range(NBS):
        xbt_q.append(
            np.ascontiguousarray(xb[q * BC : (q + 1) * BC, 1:].T).astype(bf16)
        )

    in_maps = []
    for core in range(N_CORES):
        q, h = core % NBS, core // NBS
        a_blob = np.concatenate([xbt_q[q], es_h[h][:, 0 : 2 * 128]], axis=1)
        b_blob = np.ascontiguousarray(es_h[h][:, 2 * 128 :])
        in_maps.append(
            {
                "a": np.ascontiguousarray(a_blob),
                "b": b_blob,
                "c": wh_h[h],
            }
        )
    return in_maps, xb, host_out


def kernel(x, weights, comb_idx):
    global LAST_RESULTS
    from concourse import bass_utils

    if "nc" not in _CACHE:
        _CACHE["nc"] = _build_bass()
    nc = _CACHE["nc"]

    in_maps, xb, host_out = _host_prep(x, weights, comb_idx)
    res = bass_utils.run_bass_kernel_spmd(
        nc,
        in_maps,
        core_ids=list(range(N_CORES)),
        trace=bool(int(os.environ.get("HONU_TRACE", "0"))),
    )
    LAST_RESULTS = res

    out = host_out.copy()
    for core in range(N_CORES):
        q = core % NBS
        zt = np.asarray(res.results[core]["out"], np.float64)  # [128, 4*65]
        zt = zt.reshape(128, NCH, NF)
        for ch in range(NCH):
            rows = slice(q * BC + ch * CHB, q * BC + (ch + 1) * CHB)
            out[rows] += np.sum(zt[:, ch, :] * xb[rows], axis=1)
    return out.reshape(BATCH, 1).astype(np.float32)

